# revision 5
# baseline (speedup 1.0000x reference)
"""Trainium2 Bass kernel for single-head causal attention.

Problem: x[4,2048,1024] f32; Wq/Wk/Wv [1024,1024] (torch Linear layout, y = x@W.T).
  q,k,v = x@W.T ; scores = q@k.T (causal masked, scaled 1/sqrt(1024)) ;
  out = softmax(scores)@v.

Sharding: 2 cores per batch (4 batches x 2 = 8 cores). Within a batch the 16
query blocks of 128 rows are split zig-zag so both cores get identical work
AND an identical program structure: core h=0 gets blocks [0,15,2,13,4,11,6,9],
h=1 gets [1,14,3,12,5,10,7,8]. Both orderings give causal key extents of
[1,8,2,7,3,6,4,5] chunks of 256 keys per slot, so a single SPMD program works
for all 8 cores; per-core data (x slices, gathered query rows, causal masks)
is prepared on the host.

Device pipeline per core (all dims compile-time):
  phase 1: stream xT in 4 chunks of 512 seq positions; matmul (fp32r, full
    PE rate at N>=256) against pre-transposed Wk/Wv to produce kT[e,t] (bf16)
    and v[t,e] (f32); then 2 chunks of gathered-query xqT against Wq -> qT
    (bf16).
  phase 2: per query slot j (128 queries): scores chunks of 256 keys in PSUM
    (bf16 matmul), host mask added on the causal edge chunk, row-max ->
    exp((s - max)/32) on ACT with accumulated row sum, PE-transpose of the
    weight blocks, PV matmul (fp32r) accumulating over key blocks, final
    1/sum scaling fused into the PSUM->SBUF copy, DMA out.
"""

from contextlib import ExitStack

import numpy as np

import concourse.bass as bass
import concourse.mybir as mybir
import concourse.tile as tile
from concourse import bacc
from concourse.bass_utils import run_bass_kernel_spmd
from concourse.masks import make_identity

B, S, D, E = 4, 2048, 1024, 1024
P = 128
N_CORES = 8
DT = D // P          # 8 d-tiles (contraction)
ET = E // P          # 8 e-tiles
TCH = 512            # seq chunk for projection streaming
NTC = S // TCH       # 4
SQ = S // 2          # 1024 query rows per core
NQC = SQ // TCH      # 2
KCH = 256            # key chunk for attention scores
NSLOT = SQ // P      # 8 query slots per core

# zig-zag query-block assignment: both cores' slots have identical causal
# chunk counts CJ, so one SPMD program serves all cores.
QBLOCKS = [[0, 15, 2, 13, 4, 11, 6, 9], [1, 14, 3, 12, 5, 10, 7, 8]]
CJ = [(b + 1 + 1) // 2 for b in QBLOCKS[0]]  # [1,8,2,7,3,6,4,5]
assert CJ == [(b + 1 + 1) // 2 for b in QBLOCKS[1]]

F32 = mybir.dt.float32
F32R = mybir.dt.float32r
BF16 = mybir.dt.bfloat16
AX = mybir.AxisListType.X
EXP = mybir.ActivationFunctionType.Exp
INV_SQRT_DK = 1.0 / 32.0
MASK_VAL = -1.0e9


def r(ap):
    return ap.bitcast(F32R)


def build_kernel():
    nc = bacc.Bacc(
        "TRN2",
        target_bir_lowering=False,
        debug=False,
        num_devices=N_CORES,
        dynamic_dma_scratch_size=64,
    )
    xT_d = nc.dram_tensor("xT", [NTC, P, DT, TCH], F32R, kind="ExternalInput")
    xqT_d = nc.dram_tensor("xqT", [NQC, P, DT, TCH], F32R, kind="ExternalInput")
    wq_d = nc.dram_tensor("WqT", [P, DT, E], F32R, kind="ExternalInput")
    wk_d = nc.dram_tensor("WkT", [P, DT, E], F32R, kind="ExternalInput")
    wv_d = nc.dram_tensor("WvT", [P, DT, E], F32R, kind="ExternalInput")
    msk_d = nc.dram_tensor("masks", [P, NSLOT, KCH], F32, kind="ExternalInput")
    out_d = nc.dram_tensor("out", [SQ, E], F32, kind="ExternalOutput")

    with tile.TileContext(nc) as tc, ExitStack() as ctx:
        # persistent tensors (right side): qkv projections + masks
        kqv = ctx.enter_context(tc.tile_pool(name="kqv", bufs=1, side="right"))
        kT = kqv.tile([P, ET, S], BF16, tag="kT")        # kT[p,e_t,t]: k^T
        qT = kqv.tile([P, ET, SQ], BF16, tag="qT")       # q^T, gathered rows
        vv = kqv.tile([P, S // P, E], F32R, tag="v")      # v[p,kb,e]
        msk = kqv.tile([P, NSLOT, KCH], F32, tag="msk")
        nc.sync.dma_start(msk[:], msk_d[:])

        # ---------------- phase 1: projections ----------------
        with (
            tc.tile_pool(name="wpool", bufs=2) as wpool,
            tc.tile_pool(name="xpool", bufs=2) as xpool,
            tc.tile_pool(name="pps", bufs=4, space="PSUM") as pps,
        ):
            wk_sb = wpool.tile([P, DT, E], F32R, tag="W")
            nc.sync.dma_start(wk_sb[:], wk_d[:])
            wv_sb = wpool.tile([P, DT, E], F32R, tag="W")
            nc.sync.dma_start(wv_sb[:], wv_d[:])

            for c in range(NTC):
                xc = xpool.tile([P, DT, TCH], F32R, tag="x")
                nc.sync.dma_start(xc[:], xT_d[c])
                # kT chunk: [e_t, t] accumulated over d
                for e_t in range(ET):
                    ps = pps.tile([P, TCH], F32, tag="ps")
                    for d in range(DT):
                        nc.tensor.matmul(
                            ps[:],
                            lhsT=wk_sb[:, d, e_t * P : (e_t + 1) * P],
                            rhs=xc[:, d, :],
                            start=(d == 0),
                            stop=(d == DT - 1),
                        )
                    nc.scalar.copy(kT[:, e_t, c * TCH : (c + 1) * TCH], ps[:])
                # v chunk: [t_blk, e] accumulated over d
                for tb in range(TCH // P):
                    kb = c * (TCH // P) + tb
                    for ec in range(2):
                        ps = pps.tile([P, TCH], F32, tag="ps")
                        for d in range(DT):
                            nc.tensor.matmul(
                                ps[:],
                                lhsT=xc[:, d, tb * P : (tb + 1) * P],
                                rhs=wv_sb[:, d, ec * 512 : (ec + 1) * 512],
                                start=(d == 0),
                                stop=(d == DT - 1),
                            )
                        nc.vector.tensor_copy(
                            vv[:, kb, ec * 512 : (ec + 1) * 512], ps[:]
                        )

            wq_sb = wpool.tile([P, DT, E], F32R, tag="W")
            nc.sync.dma_start(wq_sb[:], wq_d[:])
            for c in range(NQC):
                xqc = xpool.tile([P, DT, TCH], F32R, tag="x")
                nc.sync.dma_start(xqc[:], xqT_d[c])
                for e_t in range(ET):
                    ps = pps.tile([P, TCH], F32, tag="ps")
                    for d in range(DT):
                        nc.tensor.matmul(
                            ps[:],
                            lhsT=wq_sb[:, d, e_t * P : (e_t + 1) * P],
                            rhs=xqc[:, d, :],
                            start=(d == 0),
                            stop=(d == DT - 1),
                        )
                    nc.scalar.copy(qT[:, e_t, c * TCH : (c + 1) * TCH], ps[:])

        # ---------------- phase 2: attention ----------------
        with (
            tc.tile_pool(name="apool", bufs=2) as apool,
            tc.tile_pool(name="wtpool", bufs=3) as wtpool,
            tc.tile_pool(name="stpool", bufs=2) as stpool,
            tc.tile_pool(name="c1pool", bufs=1) as c1pool,
            tc.tile_pool(name="qkps", bufs=2, space="PSUM") as qkps,
            tc.tile_pool(name="pvps", bufs=4, space="PSUM") as pvps,
            tc.tile_pool(name="trps", bufs=2, space="PSUM") as trps,
        ):
            ident = c1pool.tile([P, P], F32, tag="ident")
            make_identity(nc, ident[:])

            for j in range(NSLOT):
                C = CJ[j]
                L = C * KCH
                scores = apool.tile([P, S], F32, tag="scores")
                for c in range(C):
                    ps = qkps.tile([P, KCH], F32, tag="qk")
                    for e_t in range(ET):
                        nc.tensor.matmul(
                            ps[:],
                            lhsT=qT[:, e_t, j * P : (j + 1) * P],
                            rhs=kT[:, e_t, c * KCH : (c + 1) * KCH],
                            start=(e_t == 0),
                            stop=(e_t == ET - 1),
                        )
                    dst = scores[:, c * KCH : (c + 1) * KCH]
                    if c == C - 1:
                        # causal edge: add host-built mask (0 / -1e9)
                        nc.vector.tensor_add(dst, ps[:], msk[:, j, :])
                    elif c % 2 == 0:
                        nc.scalar.copy(dst, ps[:])
                    else:
                        nc.vector.tensor_copy(dst, ps[:])

                st = stpool.tile([P, 4], F32, tag="st")
                nc.vector.tensor_reduce(
                    st[:, 0:1], scores[:, 0:L], axis=AX, op=mybir.AluOpType.max
                )
                nc.scalar.mul(st[:, 1:2], st[:, 0:1], -INV_SQRT_DK)
                wts = apool.tile([P, S], F32, tag="wts")
                nc.scalar.activation(
                    wts[:, 0:L],
                    scores[:, 0:L],
                    EXP,
                    bias=st[:, 1:2],
                    scale=INV_SQRT_DK,
                    accum_out=st[:, 2:3],
                )
                nc.vector.reciprocal(st[:, 3:4], st[:, 2:3])

                po = [
                    pvps.tile([P, 512], F32, tag="pv", name=f"po{ec}")
                    for ec in range(2)
                ]
                nkb = L // P
                for kb in range(nkb):
                    pt = trps.tile([P, P], F32, tag="tr")
                    nc.tensor.transpose(
                        pt[:], wts[:, kb * P : (kb + 1) * P], ident[:]
                    )
                    wT = wtpool.tile([P, P], F32R, tag="wT")
                    nc.scalar.copy(wT[:], pt[:])
                    for ec in range(2):
                        nc.tensor.matmul(
                            po[ec][:],
                            lhsT=wT[:],
                            rhs=vv[:, kb, ec * 512 : (ec + 1) * 512],
                            start=(kb == 0),
                            stop=(kb == nkb - 1),
                        )
                ot = apool.tile([P, E], F32, tag="out")
                for ec in range(2):
                    nc.scalar.mul(
                        ot[:, ec * 512 : (ec + 1) * 512], po[ec][:], st[:, 3:4]
                    )
                nc.sync.dma_start(out_d[j * P : (j + 1) * P, :], ot[:])

    nc.compile()
    return nc


_NC_CACHE = None


def _get_nc():
    global _NC_CACHE
    if _NC_CACHE is None:
        _NC_CACHE = build_kernel()
    return _NC_CACHE


def _pack_inputs(x, Wq, Wk, Wv):
    """Host-side relayout: everything lands in SBUF-ready, DMA-contiguous form."""
    # W.T packed: [p, d_tile, e] = W[e, d_tile*128 + p]
    def packw(w):
        return np.ascontiguousarray(w.reshape(E, DT, P).transpose(2, 1, 0))

    wqp, wkp, wvp = packw(Wq), packw(Wk), packw(Wv)

    # causal masks per slot (identical formula for both cores' block lists)
    def packmask(blocks):
        m = np.zeros((NSLOT, P, KCH), np.float32)
        for j, blk in enumerate(blocks):
            cc = np.arange(KCH)[None, :] + (CJ[j] - 1) * KCH  # key col
            rr = np.arange(P)[:, None] + blk * P              # query row
            m[j] = np.where(cc <= rr, 0.0, MASK_VAL)
        return np.ascontiguousarray(m.transpose(1, 0, 2))     # [P, slot, KCH]

    masks = [packmask(QBLOCKS[0]), packmask(QBLOCKS[1])]

    in_maps = []
    for c in range(N_CORES):
        b, h = divmod(c, 2)
        xb = x[b]  # [S, D]
        # xT packed per chunk: [c, p, d_tile, t] = x[c*TCH + t, d_tile*128 + p]
        xt = np.ascontiguousarray(
            xb.reshape(NTC, TCH, DT, P).transpose(0, 3, 2, 1)
        )
        rows = np.concatenate(
            [np.arange(blk * P, (blk + 1) * P) for blk in QBLOCKS[h]]
        )
        xq = xb[rows]  # [SQ, D]
        xqt = np.ascontiguousarray(
            xq.reshape(NQC, TCH, DT, P).transpose(0, 3, 2, 1)
        )
        in_maps.append(
            {
                "xT": xt,
                "xqT": xqt,
                "WqT": wqp,
                "WkT": wkp,
                "WvT": wvp,
                "masks": masks[h],
            }
        )
    return in_maps


def kernel(x, Wq, Wk, Wv, _spmd_kwargs=None, _results_out=None):
    x = np.asarray(x, dtype=np.float32)
    Wq = np.asarray(Wq, dtype=np.float32)
    Wk = np.asarray(Wk, dtype=np.float32)
    Wv = np.asarray(Wv, dtype=np.float32)
    assert x.shape == (B, S, D)

    nc = _get_nc()
    in_maps = _pack_inputs(x, Wq, Wk, Wv)
    res = run_bass_kernel_spmd(
        nc, in_maps, list(range(N_CORES)), **(_spmd_kwargs or {})
    )
    if _results_out is not None:
        _results_out.append(res)

    out = np.empty((B, S, E), np.float32)
    for c in range(N_CORES):
        b, h = divmod(c, 2)
        o = res.results[c]["out"]
        for j, blk in enumerate(QBLOCKS[h]):
            out[b, blk * P : (blk + 1) * P, :] = o[j * P : (j + 1) * P, :]
    return out
